# revision 20
# baseline (speedup 1.0000x reference)
"""BinLinear TRN2 kernel.

Computes (out, associate_memory) where
  scale = mean(|W|)                       (scalar)
  out   = x @ (scale * sign(W)).T         x:[4,4096,1024] f32, W:[1024,1024] f32
  associate_memory = sign(W)              [1024,1024] f32

Sharding: data-parallel over the 16384 flattened tokens across 8 cores
(2048 tokens/core); W replicated (each core computes sign/scale itself);
the f32 sign(W) output rows are sharded 128 rows/core.

Device kernel (per core, SPMD — identical program, different inputs):
  inputs : xt    [16,128,8,128] bf16  blocked x_shard.T: xt[t,p,k,c] =
                                      x_shard[t*128+c, k*128+p]
           wt    [1024, 1024] bf16  = W.T  (sign exact in bf16; scale bias ~1e-6)
           wrows [ 128, 1024] f32   = W row slice for this core's sign output
  outputs: out   [2048, 1024] f32   = x_shard @ (scale*sign(W)).T
           sgn   [ 128, 1024] f32   = sign(wrows)

The matmul runs in bf16 (sign(W) exact; x rounds to bf16) with fp32 PSUM
accumulation; the data-dependent scale is applied in fp32 during the
PSUM->SBUF epilogue on the vector engine.

Schedule: warmup dummy matmuls un-throttle the PE clock while W streams
in; a k-outer phase accumulates the first 3 token tiles chunk-by-chunk
as W chunks land (hiding the W load behind real PE work); the remaining
13 token tiles run k-inner at the warm issue rate.  Per W chunk, the
scalar engine computes the bf16 sign tile (exact, incl. sign(0)=0) and
the vector engine folds |W| into per-partition sums — one full-width op
per engine per chunk, so neither falls behind the DMA stream.
"""

import os

import ml_dtypes
import numpy as np

B, S, D_IN, D_OUT = 4, 4096, 1024, 1024
N_CORES = 8
TOK = (B * S) // N_CORES  # 2048 tokens per core
P = 128
KI = D_IN // P   # 8 contraction tiles
TT = TOK // P    # 16 token tiles per core
OROWS = D_OUT // N_CORES  # 128 sign-output rows per core
NH = D_OUT // 512  # 2 psum-bank halves of the output features
T_PHASE1 = 2     # token tiles accumulated k-outer while W streams in
WARMUP_MMS = 7   # dummy matmuls to lift the PE HAM clock gate early

_cache = {}


def _build_nc():
    import concourse.mybir as mybir
    import concourse.tile as tile
    from concourse import bacc

    f32 = mybir.dt.float32
    bf16 = mybir.dt.bfloat16
    X = mybir.AxisListType.X

    nc = bacc.Bacc(
        "TRN2",
        target_bir_lowering=False,
        debug=False,
        enable_partition_id=False,
        monotonic_sem_count=0,
    )
    xt_d = nc.dram_tensor("xt", (TT, P, KI, P), bf16, kind="ExternalInput")
    wt_d = nc.dram_tensor("wt", (D_IN, D_OUT), bf16, kind="ExternalInput")
    wr_d = nc.dram_tensor("wrows", (OROWS, D_IN), f32, kind="ExternalInput")
    out_d = nc.dram_tensor("out", (TOK, D_OUT), f32, kind="ExternalOutput")
    sgn_d = nc.dram_tensor("sgn", (OROWS, D_IN), f32, kind="ExternalOutput")

    with tile.TileContext(nc) as tc:
        with (
            tc.tile_pool(name="xw", bufs=1) as xw,
            tc.tile_pool(name="xtp", bufs=8) as xtp,
            tc.tile_pool(name="outp", bufs=6) as outp,
            tc.tile_pool(name="mm", bufs=8, space="PSUM") as mmpool,
        ):
            wt = xw.tile([P, KI, D_OUT], bf16, tag="wt")
            st = xw.tile([P, KI, D_OUT], bf16, tag="st")
            stats = xw.tile([P, KI], f32, tag="stats")
            ones = xw.tile([P, 512], bf16, tag="ones")
            csum = xw.tile([P, 1], f32, tag="csum")
            scale = xw.tile([P, 1], f32, tag="scale")
            onesf = xw.tile([P, P], f32, tag="onesf")
            wr = xw.tile([P, D_IN], f32, tag="wr")
            sg = xw.tile([P, D_IN], f32, tag="sg")

            wt_v = wt_d[:].rearrange("(k p) o -> p k o", p=P)

            nc.vector.memset(ones[:], 1.0)
            nc.vector.memset(onesf[:], 1.0)

            def xt_load(t):
                xt_t = xtp.tile([P, KI, P], bf16, tag="xt", name=f"xt{t}")
                nc.sync.dma_start(xt_t[:], xt_d[t])
                return xt_t

            # ---- input streams (SP HWDGE ring is FIFO: order = priority) ----
            # Minimize bytes ahead of the last W chunk (it gates the dense
            # region): only the phase-1 xt tiles interleave with the early W
            # chunks.  ACT Sign is exact incl. sign(0)=0 (a bitwise trick is
            # not).
            xts = {}
            for k in range(KI):
                nc.sync.dma_start(wt[:, k, :], wt_v[:, k, :])
                if k < T_PHASE1:
                    xts[k] = xt_load(k)
                nc.scalar.sign(st[:, k, :], wt[:, k, :])
                nc.vector.reduce_sum(
                    stats[:, k : k + 1], wt[:, k, :], axis=X,
                    apply_absolute_value=True,
                )
            nc.sync.dma_start(wr[:], wr_d[:])

            # ---- PE warmup: dummy matmuls into a scratch bank ----
            # One long accumulation group (no inter-matmul WAW hazards, so no
            # semaphore round-trips between them) to keep the PE busy gap-free
            # until real work arrives, lifting the HAM clock gate early.
            wps = mmpool.tile([P, 512], f32, tag="ps", name="warmps")
            for i in range(WARMUP_MMS):
                nc.tensor.matmul(
                    wps[:], ones[:, :P], ones[:],
                    start=(i == 0), stop=(i == WARMUP_MMS - 1),
                )

            # ---- phase 1: k-outer accumulation of t=0..T_PHASE1-1 ----
            ps1 = {}
            for t in range(T_PHASE1):
                for h in range(NH):
                    ps1[(t, h)] = mmpool.tile([P, 512], f32, tag="ps",
                                              name=f"ps_{t}_{h}")

            def phase1_k(k):
                for t in range(T_PHASE1):
                    for h in range(NH):
                        nc.tensor.matmul(
                            ps1[(t, h)][:],
                            xts[t][:, k, :],
                            st[:, k, h * 512 : (h + 1) * 512],
                            start=(k == 0),
                            stop=(k == KI - 1),
                        )

            for k in range(KI - 1):
                phase1_k(k)

            # scale = sum(stats)/(D_IN*D_OUT) broadcast to all partitions via
            # a ones-matmul (cross-partition reduce+broadcast in one op);
            # placed before phase-1's last k so it can't stall the epilogues,
            # reusing the warmup bank after the dummies retire.
            nc.vector.reduce_sum(csum[:], stats[:], axis=X)
            pscale = mmpool.tile([P, 1], f32, tag="ps", name="pscale")
            nc.tensor.matmul(pscale[:], onesf[:], csum[:], start=True, stop=True)
            nc.scalar.mul(scale[:], pscale[:], 1.0 / (D_IN * D_OUT))

            phase1_k(KI - 1)

            # f32 sign output for this core's slice of W (off critical path)
            nc.scalar.sign(sg[:], wr[:])
            nc.scalar.dma_start(sgn_d[:], sg[:])

            ots = {}

            def epilogue(t, h, ps, split_dma=False):
                if h == 0:
                    ots[t] = outp.tile([P, D_OUT], f32, tag="ot", name=f"ot_{t}")
                ot = ots[t]
                osl_sb = ot[:, h * 512 : (h + 1) * 512]
                if split_dma and h == NH - 1:
                    # final tile: last half's scale-copy on ACT so it runs in
                    # parallel with DVE's first half — shortens the tail
                    nc.scalar.mul(osl_sb, ps[:], scale[:])
                else:
                    nc.vector.tensor_scalar_mul(osl_sb, ps[:], scale[:])
                osl = out_d[t * P : (t + 1) * P, h * 512 : (h + 1) * 512]
                if split_dma:
                    nc.scalar.dma_start(osl, osl_sb)
                elif h == NH - 1:
                    nc.scalar.dma_start(out_d[t * P : (t + 1) * P, :], ot[:])

            for t in range(T_PHASE1):
                for h in range(NH):
                    epilogue(t, h, ps1[(t, h)])

            # ---- phase 2: k-inner over the remaining token tiles ----
            for t in range(T_PHASE1, TT):
                xt_t = xt_load(t)
                for h in range(NH):
                    ps = mmpool.tile([P, 512], f32, tag="ps", name=f"ps_{t}_{h}")
                    for k in range(KI):
                        nc.tensor.matmul(
                            ps[:],
                            xt_t[:, k, :],
                            st[:, k, h * 512 : (h + 1) * 512],
                            start=(k == 0),
                            stop=(k == KI - 1),
                        )
                    epilogue(t, h, ps, split_dma=(t == TT - 1))

    nc.compile()
    return nc


def _get_nc():
    nc = _cache.get("nc")
    if nc is None:
        nc = _cache["nc"] = _build_nc()
    return nc


LAST_RESULT = None


def kernel(x, W):
    global LAST_RESULT
    from concourse.bass_utils import run_bass_kernel_spmd

    nc = _get_nc()

    xf = np.ascontiguousarray(np.asarray(x), dtype=np.float32).reshape(B * S, D_IN)
    Wf = np.ascontiguousarray(np.asarray(W), dtype=np.float32)
    wt = Wf.T.astype(ml_dtypes.bfloat16, order="C")

    in_maps = []
    for c in range(N_CORES):
        xs = xf[c * TOK : (c + 1) * TOK, :]
        # xt[t, p, k, c] = xs[t*128 + c, k*128 + p]
        xt_c = (
            xs.reshape(TT, P, KI, P)
            .transpose(0, 3, 2, 1)
            .astype(ml_dtypes.bfloat16, order="C")
        )
        wr_c = np.ascontiguousarray(Wf[c * OROWS : (c + 1) * OROWS, :])
        in_maps.append({"xt": xt_c, "wt": wt, "wrows": wr_c})

    res = run_bass_kernel_spmd(
        nc,
        in_maps,
        core_ids=list(range(N_CORES)),
        trace=bool(os.environ.get("BASS_TRACE")),
    )
    LAST_RESULT = res
    outs = res.results
    out = np.concatenate([np.asarray(outs[c]["out"]) for c in range(N_CORES)], axis=0)
    assoc = np.concatenate(
        [np.asarray(outs[c]["sgn"]) for c in range(N_CORES)], axis=0
    )
    return out.reshape(B, S, D_OUT), assoc


# revision 22
# speedup vs baseline: 1.0272x; 1.0272x over previous
"""BinLinear TRN2 kernel.

Computes (out, associate_memory) where
  scale = mean(|W|)                       (scalar)
  out   = x @ (scale * sign(W)).T         x:[4,4096,1024] f32, W:[1024,1024] f32
  associate_memory = sign(W)              [1024,1024] f32

Sharding: data-parallel over the 16384 flattened tokens across 8 cores
(2048 tokens/core); W replicated (each core computes sign/scale itself);
the f32 sign(W) output rows are sharded 128 rows/core.

Device kernel (per core, SPMD — identical program, different inputs):
  inputs : xt    [16,128,8,128] bf16  blocked x_shard.T: xt[t,p,k,c] =
                                      x_shard[t*128+c, k*128+p]
           wt    [1024, 1024] bf16  = W.T  (sign exact in bf16; scale bias ~1e-6)
           wrows [ 128, 1024] f32   = W row slice for this core's sign output
  outputs: out   [2048, 1024] f32   = x_shard @ (scale*sign(W)).T
           sgn   [ 128, 1024] f32   = sign(wrows)

The matmul runs in bf16 (sign(W) exact; x rounds to bf16) with fp32 PSUM
accumulation; the data-dependent scale is applied in fp32 during the
PSUM->SBUF epilogue on the vector engine.

Schedule: warmup dummy matmuls un-throttle the PE clock while W streams
in; a k-outer phase accumulates the first 3 token tiles chunk-by-chunk
as W chunks land (hiding the W load behind real PE work); the remaining
13 token tiles run k-inner at the warm issue rate.  Per W chunk, the
scalar engine computes the bf16 sign tile (exact, incl. sign(0)=0) and
the vector engine folds |W| into per-partition sums — one full-width op
per engine per chunk, so neither falls behind the DMA stream.
"""

import os

import ml_dtypes
import numpy as np

B, S, D_IN, D_OUT = 4, 4096, 1024, 1024
N_CORES = 8
TOK = (B * S) // N_CORES  # 2048 tokens per core
P = 128
KI = D_IN // P   # 8 contraction tiles
TT = TOK // P    # 16 token tiles per core
OROWS = D_OUT // N_CORES  # 128 sign-output rows per core
NH = D_OUT // 512  # 2 psum-bank halves of the output features
T_PHASE1 = 3     # token tiles accumulated k-outer while W streams in
WARMUP_MMS = 7   # dummy matmuls to lift the PE HAM clock gate early

_cache = {}


def _build_nc():
    import concourse.mybir as mybir
    import concourse.tile as tile
    from concourse import bacc

    f32 = mybir.dt.float32
    bf16 = mybir.dt.bfloat16
    X = mybir.AxisListType.X

    nc = bacc.Bacc(
        "TRN2",
        target_bir_lowering=False,
        debug=False,
        enable_partition_id=False,
        monotonic_sem_count=0,
    )
    xt_d = nc.dram_tensor("xt", (TT, P, KI, P), bf16, kind="ExternalInput")
    wt_d = nc.dram_tensor("wt", (D_IN, D_OUT), bf16, kind="ExternalInput")
    wr_d = nc.dram_tensor("wrows", (OROWS, D_IN), f32, kind="ExternalInput")
    out_d = nc.dram_tensor("out", (TOK, D_OUT), f32, kind="ExternalOutput")
    sgn_d = nc.dram_tensor("sgn", (OROWS, D_IN), f32, kind="ExternalOutput")

    with tile.TileContext(nc) as tc:
        with (
            tc.tile_pool(name="xw", bufs=1) as xw,
            tc.tile_pool(name="xtp", bufs=8) as xtp,
            tc.tile_pool(name="outp", bufs=6) as outp,
            tc.tile_pool(name="mm", bufs=8, space="PSUM") as mmpool,
        ):
            wt = xw.tile([P, KI, D_OUT], bf16, tag="wt")
            st = xw.tile([P, KI, D_OUT], bf16, tag="st")
            stats = xw.tile([P, KI], f32, tag="stats")
            ones = xw.tile([P, 512], bf16, tag="ones")
            csum = xw.tile([P, 1], f32, tag="csum")
            scale = xw.tile([P, 1], f32, tag="scale")
            onesf = xw.tile([P, P], f32, tag="onesf")
            wr = xw.tile([P, D_IN], f32, tag="wr")
            sg = xw.tile([P, D_IN], f32, tag="sg")

            wt_v = wt_d[:].rearrange("(k p) o -> p k o", p=P)

            nc.vector.memset(ones[:], 1.0)
            nc.vector.memset(onesf[:], 1.0)

            def xt_load(t):
                xt_t = xtp.tile([P, KI, P], bf16, tag="xt", name=f"xt{t}")
                nc.sync.dma_start(xt_t[:], xt_d[t])
                return xt_t

            # ---- input streams (SP HWDGE ring is FIFO: order = priority) ----
            # W chunk 0 is split in halves so the first sign (and first real
            # matmul) can start ~1us earlier; phase-1 xt tiles interleave with
            # the early W chunks.  ACT Sign is exact incl. sign(0)=0 (a
            # bitwise trick is not).
            xts = {}
            for h in range(NH):
                sl = slice(h * 512, (h + 1) * 512)
                nc.sync.dma_start(wt[:, 0, sl], wt_v[:, 0, sl])
                nc.scalar.sign(st[:, 0, sl], wt[:, 0, sl])
            nc.vector.reduce_sum(
                stats[:, 0:1], wt[:, 0, :], axis=X, apply_absolute_value=True
            )
            for k in range(1, KI):
                if k - 1 < T_PHASE1:
                    xts[k - 1] = xt_load(k - 1)
                nc.sync.dma_start(wt[:, k, :], wt_v[:, k, :])
                nc.scalar.sign(st[:, k, :], wt[:, k, :])
                nc.vector.reduce_sum(
                    stats[:, k : k + 1], wt[:, k, :], axis=X,
                    apply_absolute_value=True,
                )
            nc.sync.dma_start(wr[:], wr_d[:])

            # ---- PE warmup: dummy matmuls into a scratch bank ----
            # One long accumulation group (no inter-matmul WAW hazards, so no
            # semaphore round-trips between them) to keep the PE busy gap-free
            # until real work arrives, lifting the HAM clock gate early.
            wps = mmpool.tile([P, 512], f32, tag="ps", name="warmps")
            for i in range(WARMUP_MMS):
                nc.tensor.matmul(
                    wps[:], ones[:, :P], ones[:],
                    start=(i == 0), stop=(i == WARMUP_MMS - 1),
                )

            # ---- phase 1: k-outer accumulation of t=0..T_PHASE1-1 ----
            ps1 = {}
            for t in range(T_PHASE1):
                for h in range(NH):
                    ps1[(t, h)] = mmpool.tile([P, 512], f32, tag="ps",
                                              name=f"ps_{t}_{h}")

            def phase1_k(k):
                for t in range(T_PHASE1):
                    for h in range(NH):
                        nc.tensor.matmul(
                            ps1[(t, h)][:],
                            xts[t][:, k, :],
                            st[:, k, h * 512 : (h + 1) * 512],
                            start=(k == 0),
                            stop=(k == KI - 1),
                        )

            for k in range(KI - 1):
                phase1_k(k)

            # scale = sum(stats)/(D_IN*D_OUT) broadcast to all partitions via
            # a ones-matmul (cross-partition reduce+broadcast in one op);
            # placed before phase-1's last k so it can't stall the epilogues,
            # reusing the warmup bank after the dummies retire.
            nc.vector.reduce_sum(csum[:], stats[:], axis=X)
            pscale = mmpool.tile([P, 1], f32, tag="ps", name="pscale")
            nc.tensor.matmul(pscale[:], onesf[:], csum[:], start=True, stop=True)
            nc.scalar.mul(scale[:], pscale[:], 1.0 / (D_IN * D_OUT))

            phase1_k(KI - 1)

            # f32 sign output for this core's slice of W (off critical path)
            nc.scalar.sign(sg[:], wr[:])
            nc.scalar.dma_start(sgn_d[:], sg[:])

            ots = {}

            def epilogue(t, h, ps, split_dma=False):
                if h == 0:
                    ots[t] = outp.tile([P, D_OUT], f32, tag="ot", name=f"ot_{t}")
                ot = ots[t]
                osl_sb = ot[:, h * 512 : (h + 1) * 512]
                if split_dma and h == NH - 1:
                    # final tile: last half's scale-copy on ACT so it runs in
                    # parallel with DVE's first half — shortens the tail
                    nc.scalar.mul(osl_sb, ps[:], scale[:])
                else:
                    nc.vector.tensor_scalar_mul(osl_sb, ps[:], scale[:])
                osl = out_d[t * P : (t + 1) * P, h * 512 : (h + 1) * 512]
                if split_dma:
                    nc.scalar.dma_start(osl, osl_sb)
                elif h == NH - 1:
                    nc.scalar.dma_start(out_d[t * P : (t + 1) * P, :], ot[:])

            for t in range(T_PHASE1):
                for h in range(NH):
                    epilogue(t, h, ps1[(t, h)])

            # ---- phase 2: k-inner over the remaining token tiles ----
            for t in range(T_PHASE1, TT):
                xt_t = xt_load(t)
                for h in range(NH):
                    ps = mmpool.tile([P, 512], f32, tag="ps", name=f"ps_{t}_{h}")
                    for k in range(KI):
                        nc.tensor.matmul(
                            ps[:],
                            xt_t[:, k, :],
                            st[:, k, h * 512 : (h + 1) * 512],
                            start=(k == 0),
                            stop=(k == KI - 1),
                        )
                    epilogue(t, h, ps, split_dma=(t == TT - 1))

    nc.compile()
    return nc


def _get_nc():
    nc = _cache.get("nc")
    if nc is None:
        nc = _cache["nc"] = _build_nc()
    return nc


LAST_RESULT = None


def kernel(x, W):
    global LAST_RESULT
    from concourse.bass_utils import run_bass_kernel_spmd

    nc = _get_nc()

    xf = np.ascontiguousarray(np.asarray(x), dtype=np.float32).reshape(B * S, D_IN)
    Wf = np.ascontiguousarray(np.asarray(W), dtype=np.float32)
    wt = Wf.T.astype(ml_dtypes.bfloat16, order="C")

    in_maps = []
    for c in range(N_CORES):
        xs = xf[c * TOK : (c + 1) * TOK, :]
        # xt[t, p, k, c] = xs[t*128 + c, k*128 + p]
        xt_c = (
            xs.reshape(TT, P, KI, P)
            .transpose(0, 3, 2, 1)
            .astype(ml_dtypes.bfloat16, order="C")
        )
        wr_c = np.ascontiguousarray(Wf[c * OROWS : (c + 1) * OROWS, :])
        in_maps.append({"xt": xt_c, "wt": wt, "wrows": wr_c})

    res = run_bass_kernel_spmd(
        nc,
        in_maps,
        core_ids=list(range(N_CORES)),
        trace=bool(os.environ.get("BASS_TRACE")),
    )
    LAST_RESULT = res
    outs = res.results
    out = np.concatenate([np.asarray(outs[c]["out"]) for c in range(N_CORES)], axis=0)
    assoc = np.concatenate(
        [np.asarray(outs[c]["sgn"]) for c in range(N_CORES)], axis=0
    )
    return out.reshape(B, S, D_OUT), assoc
